# revision 19
# baseline (speedup 1.0000x reference)
"""DGCNConv (GNN message passing) Trainium2 kernel, 8-core SPMD.

Strategy (graph/data parallel, per sharding hint):
- Nodes are partitioned into 8 contiguous ranges of 6250. Core c owns the
  dst-range edges for agg_in and the src-range edges for agg_out.
- Per direction, edges are sorted HALF-MAJOR (gather-table half, then target
  block of 128): each direction becomes two giant contiguous gather streams,
  so SWDGE dma_gather instructions are maximal-size (GCAP) and their ~1us
  fixed issue overhead is amortized (the dominant cost of the previous
  block-major layout).
- Edges are gathered from HBM with SWDGE dma_gather (256B rows, fp16
  duplicated x table) and segment-reduced on-chip with one-hot matmuls
  accumulating in PSUM (aggT layout [feat, node]).
- Per 128-node block (interleaved, not phase-by-phase): reduce in-dir
  chunks, reduce out-dir chunks, then immediately run the epilogue
  outT = W_self@xT + W_in@agg_inT + W_out@agg_outT, ReLU (+BN partial
  sums). This overlaps PE/Act/DVE with the gather stream and avoids
  materializing full-size agg buffers (SBUF was the binding constraint).
- Tail: cross-core AllReduce of BN partials, fused scale/shift normalize
  (written into the retired xT buffer), writeback.
- Host: routes/pads edges, builds index tables, transposes the output back.

The per-(half,block) segment sizes are padded to the max over cores so all
8 cores run one identical program (SPMD NEFF) on per-core data.
"""

import sys

if "/opt/trn_rl_repo" not in sys.path:
    sys.path.insert(0, "/opt/trn_rl_repo")

import numpy as np

N_NODES = 50000
N_EDGES = 800000
D = 64
N_CORES = 8
NPC = N_NODES // N_CORES          # 6250 nodes per core
NBLK = (NPC + 127) // 128         # 49 blocks per core
HALF = N_NODES // 2               # 25000, int16-safe gather base split
BN_EPS = 1e-5
GCAP = 1024                       # positions per dma_gather instruction
SCRATCH = 16384                   # SWDGE descriptor ring bytes (16B/desc)
MB = 16                           # 128-edge chunks per one-hot build batch


# ---------------------------------------------------------------- host prep

def _route_direction(t_all, g_all):
    """Route edges (t = reduce-target node id, g = gather node id) to cores.

    Returns (layout, per_core): layout is the static position map shared by
    all cores (half-major streams); per_core holds each core's idx/dcmp.
    """
    core_of = t_all // NPC
    per_core_edges = []
    for c in range(N_CORES):
        m = core_of == c
        t = t_all[m] - c * NPC
        g = g_all[m]
        blk = t >> 7
        half = (g >= HALF).astype(np.int64)
        order = np.lexsort((g, blk, half))
        per_core_edges.append((t[order], g[order], blk[order], half[order]))

    # static budgets per (half, blk): max over cores only — block boundaries
    # fall mid-chunk; a second iota bank (128..255) handles chunks shared
    # between adjacent blocks. Stream tails are padded to whole chunks.
    budgets = np.zeros((2, NBLK), np.int64)
    for c in range(N_CORES):
        t, g, blk, half = per_core_edges[c]
        cnt = np.bincount(half * NBLK + blk, minlength=2 * NBLK).reshape(2, NBLK)
        budgets = np.maximum(budgets, cnt)
    assert budgets.min() >= 128

    # layout: positions ordered half-major, then block
    seg_start = np.zeros((2, NBLK), np.int64)
    stream_bounds = []  # (start, end) per half, end 128-aligned
    pos = 0
    for h in range(2):
        s0 = pos
        for b in range(NBLK):
            seg_start[h, b] = pos
            pos += budgets[h, b]
        pos = ((pos + 127) // 128) * 128  # stream tail pad
        stream_bounds.append((s0, pos))
    total = pos
    nch = total // 128

    # gather runs: each half stream is contiguous; split at GCAP only
    runs = []  # (pos0, npos, half)
    for h in range(2):
        p0, pend = stream_bounds[h]
        while p0 < pend:
            take = min(pend - p0, GCAP)
            runs.append((p0, take, h))
            p0 += take

    # static chunk -> reference block (block containing the chunk's first
    # position); edges of block bref+1 inside the chunk compare as 128..255
    bref = np.zeros(nch, np.int64)
    for h in range(2):
        for b in range(NBLK):
            p0 = int(seg_start[h, b])
            p1 = p0 + int(budgets[h, b])
            bref[(p0 + 127) // 128:(p1 + 127) // 128] = b
    # per-block position ranges (one per half)
    blocks = []  # (blk, [(pos0, pos1), ...])
    for b in range(NBLK):
        segs = []
        for h in range(2):
            p0 = int(seg_start[h, b])
            p1 = p0 + int(budgets[h, b])
            segs.append((p0, p1))
        blocks.append((b, segs))

    # chunk -> run mapping
    chunk_run = np.zeros(nch, np.int64)
    run_c0 = np.zeros(len(runs), np.int64)
    for ri, (p0, n, h) in enumerate(runs):
        run_c0[ri] = p0 // 128
        chunk_run[p0 // 128:(p0 + n) // 128] = ri

    # per-core position arrays
    per_core = []
    for c in range(N_CORES):
        t, g, blk, half = per_core_edges[c]
        idx = np.zeros(total, np.int16)          # gather idx rel to half base
        dcmp = np.full(total, -1.0, np.float16)  # one-hot compare value
        key = half * NBLK + blk
        cnt = np.bincount(key, minlength=2 * NBLK)
        estart = np.zeros(2 * NBLK, np.int64)
        estart[1:] = np.cumsum(cnt)[:-1]
        for h in range(2):
            for b in range(NBLK):
                n = int(cnt[h * NBLK + b])
                if n == 0:
                    continue
                e0 = int(estart[h * NBLK + b])
                p0 = int(seg_start[h, b])
                p = p0 + np.arange(n)
                rel = t[e0:e0 + n] - bref[p // 128] * 128
                assert rel.min() >= 0 and rel.max() < 256
                idx[p0:p0 + n] = (g[e0:e0 + n] - h * HALF).astype(np.int16)
                dcmp[p0:p0 + n] = rel.astype(np.float16)
        idx_wrapped = np.tile(
            np.ascontiguousarray(idx.reshape(-1, 16).T), (8, 1))
        dcmp_wrapped = np.ascontiguousarray(dcmp.reshape(-1, 128).T)
        per_core.append((idx_wrapped, dcmp_wrapped))

    layout = dict(total=total, nch=nch, runs=runs, blocks=blocks,
                  chunk_run=chunk_run, run_c0=run_c0)
    return layout, per_core


# ---------------------------------------------------------------- program

def _build_program(lay_in, lay_out):
    import concourse.bacc as bacc
    import concourse.mybir as mybir
    from concourse import tile
    from concourse import library_config

    f32, f16, i16 = mybir.dt.float32, mybir.dt.float16, mybir.dt.int16
    nc = bacc.Bacc(None, target_bir_lowering=False, debug=False,
                   dynamic_dma_scratch_size=SCRATCH, num_swdge_queues=4)

    xdup = nc.dram_tensor("xdup", [N_NODES, 2 * D], f16, kind="ExternalInput")
    xT_d = nc.dram_tensor("xT", [D, NPC], f32, kind="ExternalInput")
    Wt_d = nc.dram_tensor("Wt", [D, 3 * D], f32, kind="ExternalInput")
    gb_d = nc.dram_tensor("gb", [D, 2], f32, kind="ExternalInput")
    out_d = nc.dram_tensor("out", [D, NPC], f32, kind="ExternalOutput")
    cc_in = nc.dram_tensor("cc_in", [D, 2], f32)
    cc_out = nc.dram_tensor("cc_out", [D, 2], f32, addr_space="Shared")

    lays = {"in": lay_in, "out": lay_out}
    idx_d, dcmp_d = {}, {}
    for dk in ("in", "out"):
        tot = lays[dk]["total"]
        idx_d[dk] = nc.dram_tensor(
            f"idx_{dk}", [128, tot // 16], i16, kind="ExternalInput")
        dcmp_d[dk] = nc.dram_tensor(
            f"dcmp_{dk}", [128, tot // 128], f16, kind="ExternalInput")

    with tile.TileContext(nc) as tc:
        nc.gpsimd.load_library(library_config.mlp)
        with (
            tc.tile_pool(name="const", bufs=1) as cpool,
            tc.tile_pool(name="gath", bufs=16) as gpool,
            tc.tile_pool(name="mb", bufs=8) as mpool,
            tc.tile_pool(name="dr", bufs=4) as dpool,
            tc.tile_pool(name="aggt", bufs=4) as apool,
            tc.tile_pool(name="agg_ps", bufs=4, space="PSUM") as agg_pspool,
            tc.tile_pool(name="out_ps", bufs=3, space="PSUM") as out_pspool,
        ):
            # --- constants
            xT = cpool.tile([D, NPC], f32, tag="xT")
            nc.sync.dma_start(xT[:], xT_d[:])
            Wt = cpool.tile([D, 3 * D], f32, tag="Wt")
            nc.sync.dma_start(Wt[:], Wt_d[:])
            gb = cpool.tile([D, 2], f32, tag="gb")
            nc.sync.dma_start(gb[:], gb_d[:])
            iota_i = cpool.tile([128, MB, 128], i16, tag="iota_i")
            nc.gpsimd.iota(iota_i[:], [[0, MB], [1, 128]], base=0,
                           channel_multiplier=0)
            iota_f = cpool.tile([128, MB, 128], f16, tag="iota_f")
            nc.vector.tensor_copy(iota_f[:], iota_i[:])
            iota1_i = cpool.tile([128, 128], i16, tag="iota1_i")
            nc.gpsimd.iota(iota1_i[:], [[1, 128]], base=128,
                           channel_multiplier=0)
            iota1_f = cpool.tile([128, 128], f16, tag="iota1_f")
            nc.vector.tensor_copy(iota1_f[:], iota1_i[:])

            idx_t, dcmp_t = {}, {}
            for dk in ("in", "out"):
                tot = lays[dk]["total"]
                idx_t[dk] = cpool.tile([128, tot // 16], i16, tag=f"idx{dk}",
                                       name=f"idx_t_{dk}")
                nc.sync.dma_start(idx_t[dk][:], idx_d[dk][:])
                dcmp_t[dk] = cpool.tile([128, tot // 128], f16, tag=f"dc{dk}",
                                        name=f"dcmp_t_{dk}")
                nc.sync.dma_start(dcmp_t[dk][:], dcmp_d[dk][:])

            g_tiles = {"in": {}, "out": {}}
            m_tiles = {"in": {}, "out": {}}
            _bcast_failed = []

            def emit_run(dk, ri):
                lay = lays[dk]
                p0, npos, h = lay["runs"][ri]
                gt = gpool.tile([128, npos // 128, 2 * D], f16, tag="g",
                                name=f"g_{dk}_{ri}")
                src = xdup[h * HALF:(h + 1) * HALF, :]
                qn = (0 if dk == "in" else 2) + h  # one queue per stream
                nc.gpsimd.dma_gather(
                    gt[:], src, idx_t[dk][:, p0 // 16:(p0 + npos) // 16],
                    npos, npos, 2 * D, queue_num=qn)
                g_tiles[dk][ri] = gt

            def emit_mask(dk, c0, c1, bank):
                """One-hot masks for chunks [c0, c1) vs iota bank 0 or 1."""
                nb = c1 - c0
                assert 0 < nb <= MB, (c0, c1)
                bcast = dcmp_t[dk][:, c0:c0 + nb].unsqueeze(2) \
                    .broadcast_to([128, nb, 128])
                mt = mpool.tile([128, nb, 128], f16, tag="m",
                                name=f"m_{dk}_{c0}_{bank}")
                ref = iota_f[:, :nb, :] if bank == 0 \
                    else iota1_f[:].unsqueeze(1).broadcast_to([128, nb, 128])
                nc.vector.tensor_tensor(mt[:], ref, bcast,
                                        op=mybir.AluOpType.is_equal)
                return mt

            # --- per-block: segment-reduce both directions, then epilogue
            r_sb = cpool.tile([D, NPC], f32, tag="r")
            sums = cpool.tile([D, NBLK], f32, tag="sums")
            sumsq = cpool.tile([D, NBLK], f32, tag="sumsq")
            sq_scr = cpool.tile([D, 128], f32, tag="sq")

            for b in range(NBLK):
                agg_t = {}
                for dk in ("in", "out"):
                    lay = lays[dk]
                    _, segs = lay["blocks"][b]
                    at = apool.tile([D, 128], f32, tag="aggt",
                                    name=f"at_{dk}_{b}")
                    if not segs:
                        nc.vector.memset(at[:], 0.0)
                        agg_t[dk] = at
                        continue
                    # (mask_tile, col, chunk) worklist over both halves
                    work = []
                    for (pos0, pos1) in segs:
                        k0 = pos0 // 128
                        kend = (pos1 + 127) // 128
                        kb = k0
                        if pos0 % 128 != 0:
                            # shared boundary chunk: edges of this block sit
                            # at 128..255 relative to the chunk's ref block
                            m1 = emit_mask(dk, k0, k0 + 1, 1)
                            work.append((m1, 0, k0))
                            kb = k0 + 1
                        if kend > kb:
                            m0 = emit_mask(dk, kb, kend, 0)
                            for c in range(kb, kend):
                                work.append((m0, c - kb, c))
                    aps = agg_pspool.tile([D, 128], f32, tag="aggps",
                                          name=f"aps_{dk}_{b}")
                    for j, (mt, col, c) in enumerate(work):
                        ri = int(lay["chunk_run"][c])
                        if ri not in g_tiles[dk]:
                            emit_run(dk, ri)
                        nc.tensor.matmul(
                            aps[:],
                            g_tiles[dk][ri][:, c - int(lay["run_c0"][ri]), 0:D],
                            mt[:, col, :],
                            start=(j == 0), stop=(j == len(work) - 1))
                    nc.vector.tensor_copy(at[:], aps[:])
                    agg_t[dk] = at

                ncols = min(128, NPC - b * 128)
                ops = out_pspool.tile([D, ncols], f32, tag="outps",
                                      name=f"ops_{b}")
                nc.tensor.matmul(ops[:], Wt[:, 0:D],
                                 xT[:, b * 128:b * 128 + ncols],
                                 start=True, stop=False)
                nc.tensor.matmul(ops[:], Wt[:, D:2 * D],
                                 agg_t["in"][:, 0:ncols],
                                 start=False, stop=False)
                nc.tensor.matmul(ops[:], Wt[:, 2 * D:3 * D],
                                 agg_t["out"][:, 0:ncols],
                                 start=False, stop=True)
                nc.scalar.activation(
                    r_sb[:, b * 128:b * 128 + ncols], ops[:],
                    mybir.ActivationFunctionType.Relu,
                    accum_out=sums[:, b:b + 1])
                nc.scalar.activation(
                    sq_scr[:, 0:ncols], r_sb[:, b * 128:b * 128 + ncols],
                    mybir.ActivationFunctionType.Square,
                    accum_out=sumsq[:, b:b + 1])

            # --- BN stats allreduce
            part = cpool.tile([D, 2], f32, tag="part")
            nc.vector.tensor_reduce(part[:, 0:1], sums[:],
                                    mybir.AxisListType.X, mybir.AluOpType.add)
            nc.vector.tensor_reduce(part[:, 1:2], sumsq[:],
                                    mybir.AxisListType.X, mybir.AluOpType.add)
            nc.sync.dma_start(cc_in[:], part[:])
            nc.gpsimd.collective_compute(
                "AllReduce", mybir.AluOpType.add,
                replica_groups=[list(range(N_CORES))],
                ins=[cc_in[:]], outs=[cc_out[:]])
            tot = cpool.tile([D, 2], f32, tag="tot")
            nc.sync.dma_start(tot[:], cc_out[:])

            # --- scale/shift
            stats = cpool.tile([D, 8], f32, tag="stats")
            mean, ex2 = stats[:, 0:1], stats[:, 1:2]
            var, std = stats[:, 2:3], stats[:, 3:4]
            inv, scale = stats[:, 4:5], stats[:, 5:6]
            shift, tmp = stats[:, 6:7], stats[:, 7:8]
            inv_n = 1.0 / float(N_NODES)
            nc.vector.tensor_scalar_mul(mean, tot[:, 0:1], inv_n)
            nc.vector.tensor_scalar_mul(ex2, tot[:, 1:2], inv_n)
            nc.vector.tensor_tensor(tmp, mean, mean, op=mybir.AluOpType.mult)
            nc.vector.tensor_tensor(var, ex2, tmp,
                                    op=mybir.AluOpType.subtract)
            nc.vector.tensor_scalar_add(var, var, BN_EPS)
            nc.scalar.activation(std, var, mybir.ActivationFunctionType.Sqrt)
            nc.vector.reciprocal(inv, std)
            nc.vector.tensor_tensor(scale, gb[:, 0:1], inv,
                                    op=mybir.AluOpType.mult)
            nc.vector.tensor_tensor(tmp, mean, scale,
                                    op=mybir.AluOpType.mult)
            nc.vector.tensor_tensor(shift, gb[:, 1:2], tmp,
                                    op=mybir.AluOpType.subtract)

            # --- normalize into the retired xT buffer + writeback
            nc.vector.tensor_scalar(xT[:], r_sb[:], scale, shift,
                                    op0=mybir.AluOpType.mult,
                                    op1=mybir.AluOpType.add)
            nc.sync.dma_start(out_d[:], xT[:])

    if _bcast_failed:
        print(f"note: mask broadcast tensor_tensor fell back to copy "
              f"for {len(_bcast_failed)} batches", file=sys.stderr)
    nc.finalize()
    return nc


# ---------------------------------------------------------------- kernel

def prepare(x, edge_index, num_nodes=None, W_in=None, W_out=None,
            W_self=None, gamma=None, beta=None):
    """Build the bass program and per-core input maps."""
    x = np.asarray(x, np.float32)
    edge_index = np.asarray(edge_index, np.int64)
    W_in = np.asarray(W_in, np.float32)
    W_out = np.asarray(W_out, np.float32)
    W_self = np.asarray(W_self, np.float32)
    gamma = np.asarray(gamma, np.float32)
    beta = np.asarray(beta, np.float32)
    assert x.shape == (N_NODES, D) and edge_index.shape == (2, N_EDGES)

    src, dst = edge_index[0], edge_index[1]
    lay_in, pc_in = _route_direction(dst, src)    # agg_in: reduce by dst
    lay_out, pc_out = _route_direction(src, dst)  # agg_out: reduce by src

    nc = _build_program(lay_in, lay_out)

    x16 = x.astype(np.float16)
    xdup = np.concatenate([x16, x16], axis=1)
    Wt = np.concatenate([W_self.T, W_in.T, W_out.T], axis=1).astype(np.float32)
    Wt = np.ascontiguousarray(Wt)
    gb = np.ascontiguousarray(np.stack([gamma, beta], axis=1).astype(np.float32))

    in_maps = []
    for c in range(N_CORES):
        xT_c = np.ascontiguousarray(x[c * NPC:(c + 1) * NPC].T)
        in_maps.append({
            "xdup": xdup,
            "xT": xT_c,
            "Wt": Wt,
            "gb": gb,
            "idx_in": pc_in[c][0], "dcmp_in": pc_in[c][1],
            "idx_out": pc_out[c][0], "dcmp_out": pc_out[c][1],
        })
    return nc, in_maps


def postprocess(results):
    outT = np.concatenate([r["out"] for r in results], axis=1)
    return np.ascontiguousarray(outT.T).astype(np.float32)


def kernel(x, edge_index, num_nodes=None, W_in=None, W_out=None,
           W_self=None, gamma=None, beta=None):
    from concourse.bass_utils import run_bass_kernel_spmd

    nc, in_maps = prepare(x, edge_index, num_nodes, W_in, W_out,
                          W_self, gamma, beta)
    res = run_bass_kernel_spmd(nc, in_maps, core_ids=list(range(N_CORES)))
    return postprocess(res.results)


# revision 20
# speedup vs baseline: 1.9150x; 1.9150x over previous
"""DGCNConv (GNN message passing) Trainium2 kernel, 8-core SPMD.

Strategy (graph/data parallel, per sharding hint):
- Nodes are partitioned into 8 contiguous ranges of 6250. Core c owns the
  dst-range edges for agg_in and the src-range edges for agg_out.
- Per direction, edges are sorted HALF-MAJOR (gather-table half, then target
  block of 128): each direction becomes two giant contiguous gather streams,
  so SWDGE dma_gather instructions are maximal-size (GCAP) and their ~1us
  fixed issue overhead is amortized (the dominant cost of the previous
  block-major layout).
- Edges are gathered from HBM with SWDGE dma_gather (256B rows, fp16
  duplicated x table) and segment-reduced on-chip with one-hot matmuls
  accumulating in PSUM (aggT layout [feat, node]).
- Per 128-node block (interleaved, not phase-by-phase): reduce in-dir
  chunks, reduce out-dir chunks, then immediately run the epilogue
  outT = W_self@xT + W_in@agg_inT + W_out@agg_outT, ReLU (+BN partial
  sums). This overlaps PE/Act/DVE with the gather stream and avoids
  materializing full-size agg buffers (SBUF was the binding constraint).
- Tail: cross-core AllReduce of BN partials, fused scale/shift normalize
  (written into the retired xT buffer), writeback.
- Host: routes/pads edges, builds index tables, transposes the output back.

The per-(half,block) segment sizes are padded to the max over cores so all
8 cores run one identical program (SPMD NEFF) on per-core data.
"""

import sys

if "/opt/trn_rl_repo" not in sys.path:
    sys.path.insert(0, "/opt/trn_rl_repo")

import numpy as np

N_NODES = 50000
N_EDGES = 800000
D = 64
N_CORES = 8
NPC = N_NODES // N_CORES          # 6250 nodes per core
NBLK = (NPC + 127) // 128         # 49 blocks per core
HALF = N_NODES // 2               # 25000, int16-safe gather base split
BN_EPS = 1e-5
GCAP = 1024                       # positions per dma_gather instruction
SCRATCH = 16384                   # SWDGE descriptor ring bytes (16B/desc)
MB = 16                           # 128-edge chunks per one-hot build batch


# ---------------------------------------------------------------- host prep

def _route_direction(t_all, g_all):
    """Route edges (t = reduce-target node id, g = gather node id) to cores.

    Returns (layout, per_core): layout is the static position map shared by
    all cores (half-major streams); per_core holds each core's idx/dcmp.
    """
    core_of = t_all // NPC
    per_core_edges = []
    for c in range(N_CORES):
        m = core_of == c
        t = t_all[m] - c * NPC
        g = g_all[m]
        blk = t >> 7
        half = (g >= HALF).astype(np.int64)
        order = np.lexsort((g, blk, half))
        per_core_edges.append((t[order], g[order], blk[order], half[order]))

    # static budgets per (half, blk): max over cores, padded to whole chunks
    budgets = np.zeros((2, NBLK), np.int64)
    for c in range(N_CORES):
        t, g, blk, half = per_core_edges[c]
        cnt = np.bincount(half * NBLK + blk, minlength=2 * NBLK).reshape(2, NBLK)
        budgets = np.maximum(budgets, cnt)
    budgets = ((budgets + 127) // 128) * 128

    # layout: positions ordered half-major, then block
    seg_start = np.zeros((2, NBLK), np.int64)
    pos = 0
    for h in range(2):
        for b in range(NBLK):
            seg_start[h, b] = pos
            pos += budgets[h, b]
    total = pos
    nch = total // 128

    # gather runs: each half stream is contiguous; split at GCAP only
    runs = []  # (pos0, npos, half)
    for h in range(2):
        p0 = int(seg_start[h, 0])
        pend = int(seg_start[h, NBLK - 1] + budgets[h, NBLK - 1])
        while p0 < pend:
            take = min(pend - p0, GCAP)
            runs.append((p0, take, h))
            p0 += take

    # per-block chunk ranges (one per half)
    blocks = []  # (blk, [(c0, c1), ...])
    for b in range(NBLK):
        segs = []
        for h in range(2):
            c0 = int(seg_start[h, b]) // 128
            c1 = c0 + int(budgets[h, b]) // 128
            if c1 > c0:
                segs.append((c0, c1))
        blocks.append((b, segs))

    # chunk -> run mapping
    chunk_run = np.zeros(nch, np.int64)
    run_c0 = np.zeros(len(runs), np.int64)
    for ri, (p0, n, h) in enumerate(runs):
        run_c0[ri] = p0 // 128
        chunk_run[p0 // 128:(p0 + n) // 128] = ri

    # per-core position arrays
    per_core = []
    for c in range(N_CORES):
        t, g, blk, half = per_core_edges[c]
        idx = np.zeros(total, np.int16)          # gather idx rel to half base
        dcmp = np.full(total, -1.0, np.float16)  # one-hot compare value
        key = half * NBLK + blk
        cnt = np.bincount(key, minlength=2 * NBLK)
        estart = np.zeros(2 * NBLK, np.int64)
        estart[1:] = np.cumsum(cnt)[:-1]
        for h in range(2):
            for b in range(NBLK):
                n = int(cnt[h * NBLK + b])
                if n == 0:
                    continue
                e0 = int(estart[h * NBLK + b])
                p0 = int(seg_start[h, b])
                idx[p0:p0 + n] = (g[e0:e0 + n] - h * HALF).astype(np.int16)
                dcmp[p0:p0 + n] = (t[e0:e0 + n] - b * 128).astype(np.float16)
        idx_wrapped = np.tile(
            np.ascontiguousarray(idx.reshape(-1, 16).T), (8, 1))
        dcmp_wrapped = np.ascontiguousarray(dcmp.reshape(-1, 128).T)
        per_core.append((idx_wrapped, dcmp_wrapped))

    layout = dict(total=total, nch=nch, runs=runs, blocks=blocks,
                  chunk_run=chunk_run, run_c0=run_c0)
    return layout, per_core


# ---------------------------------------------------------------- program

def _build_program(lay_in, lay_out):
    import concourse.bacc as bacc
    import concourse.mybir as mybir
    from concourse import tile
    from concourse import library_config

    f32, f16, i16 = mybir.dt.float32, mybir.dt.float16, mybir.dt.int16
    nc = bacc.Bacc(None, target_bir_lowering=False, debug=False,
                   dynamic_dma_scratch_size=SCRATCH, num_swdge_queues=4)

    xdup = nc.dram_tensor("xdup", [N_NODES, 2 * D], f16, kind="ExternalInput")
    xT_d = nc.dram_tensor("xT", [D, NPC], f32, kind="ExternalInput")
    Wt_d = nc.dram_tensor("Wt", [D, 3 * D], f32, kind="ExternalInput")
    gb_d = nc.dram_tensor("gb", [D, 2], f32, kind="ExternalInput")
    out_d = nc.dram_tensor("out", [D, NPC], f32, kind="ExternalOutput")
    cc_in = nc.dram_tensor("cc_in", [D, 2], f32)
    cc_out = nc.dram_tensor("cc_out", [D, 2], f32, addr_space="Shared")

    lays = {"in": lay_in, "out": lay_out}
    idx_d, dcmp_d = {}, {}
    for dk in ("in", "out"):
        tot = lays[dk]["total"]
        idx_d[dk] = nc.dram_tensor(
            f"idx_{dk}", [128, tot // 16], i16, kind="ExternalInput")
        dcmp_d[dk] = nc.dram_tensor(
            f"dcmp_{dk}", [128, tot // 128], f16, kind="ExternalInput")

    with tile.TileContext(nc) as tc:
        nc.gpsimd.load_library(library_config.mlp)
        with (
            tc.tile_pool(name="const", bufs=1) as cpool,
            tc.tile_pool(name="gath", bufs=16) as gpool,
            tc.tile_pool(name="mb", bufs=8) as mpool,
            tc.tile_pool(name="dr", bufs=4) as dpool,
            tc.tile_pool(name="aggt", bufs=4) as apool,
            tc.tile_pool(name="agg_ps", bufs=4, space="PSUM") as agg_pspool,
            tc.tile_pool(name="out_ps", bufs=3, space="PSUM") as out_pspool,
        ):
            # --- constants
            xT = cpool.tile([D, NPC], f32, tag="xT")
            nc.sync.dma_start(xT[:], xT_d[:])
            Wt = cpool.tile([D, 3 * D], f32, tag="Wt")
            nc.sync.dma_start(Wt[:], Wt_d[:])
            gb = cpool.tile([D, 2], f32, tag="gb")
            nc.sync.dma_start(gb[:], gb_d[:])
            iota_i = cpool.tile([128, MB, 128], i16, tag="iota_i")
            nc.gpsimd.iota(iota_i[:], [[0, MB], [1, 128]], base=0,
                           channel_multiplier=0)
            iota_f = cpool.tile([128, MB, 128], f16, tag="iota_f")
            nc.vector.tensor_copy(iota_f[:], iota_i[:])

            idx_t, dcmp_t = {}, {}
            for dk in ("in", "out"):
                tot = lays[dk]["total"]
                idx_t[dk] = cpool.tile([128, tot // 16], i16, tag=f"idx{dk}",
                                       name=f"idx_t_{dk}")
                nc.sync.dma_start(idx_t[dk][:], idx_d[dk][:])
                dcmp_t[dk] = cpool.tile([128, tot // 128], f16, tag=f"dc{dk}",
                                        name=f"dcmp_t_{dk}")
                nc.sync.dma_start(dcmp_t[dk][:], dcmp_d[dk][:])

            g_tiles = {"in": {}, "out": {}}
            m_tiles = {"in": {}, "out": {}}
            _bcast_failed = []

            def emit_run(dk, ri):
                lay = lays[dk]
                p0, npos, h = lay["runs"][ri]
                gt = gpool.tile([128, npos // 128, 2 * D], f16, tag="g",
                                name=f"g_{dk}_{ri}")
                src = xdup[h * HALF:(h + 1) * HALF, :]
                qn = (0 if dk == "in" else 2) + h  # one queue per stream
                nc.gpsimd.dma_gather(
                    gt[:], src, idx_t[dk][:, p0 // 16:(p0 + npos) // 16],
                    npos, npos, 2 * D, queue_num=qn)
                g_tiles[dk][ri] = gt

            def emit_seg(dk, c0, c1):
                """Build the one-hot masks for one (half, block) segment."""
                nb = c1 - c0
                assert nb <= MB, (c0, c1)
                bcast = dcmp_t[dk][:, c0:c0 + nb].unsqueeze(2) \
                    .broadcast_to([128, nb, 128])
                mt = mpool.tile([128, nb, 128], f16, tag="m",
                                name=f"m_{dk}_{c0}")
                try:
                    nc.vector.tensor_tensor(
                        mt[:], iota_f[:, :nb, :], bcast,
                        op=mybir.AluOpType.is_equal)
                except Exception:
                    _bcast_failed.append(c0)
                    dr = dpool.tile([128, nb, 128], f16, tag="drep",
                                    name=f"dr_{dk}_{c0}")
                    nc.vector.tensor_copy(dr[:], bcast)
                    nc.vector.tensor_tensor(
                        mt[:], iota_f[:, :nb, :], dr[:],
                        op=mybir.AluOpType.is_equal)
                m_tiles[dk][c0] = mt

            # --- per-block: segment-reduce both directions, then epilogue
            r_sb = cpool.tile([D, NPC], f32, tag="r")
            sums = cpool.tile([D, NBLK], f32, tag="sums")
            sumsq = cpool.tile([D, NBLK], f32, tag="sumsq")
            sq_scr = cpool.tile([D, 128], f32, tag="sq")

            for b in range(NBLK):
                agg_t = {}
                for dk in ("in", "out"):
                    lay = lays[dk]
                    _, segs = lay["blocks"][b]
                    at = apool.tile([D, 128], f32, tag="aggt",
                                    name=f"at_{dk}_{b}")
                    if not segs:
                        nc.vector.memset(at[:], 0.0)
                        agg_t[dk] = at
                        continue
                    nchunks = sum(c1 - c0 for (c0, c1) in segs)
                    aps = agg_pspool.tile([D, 128], f32, tag="aggps",
                                          name=f"aps_{dk}_{b}")
                    j = 0
                    for (c0, c1) in segs:
                        emit_seg(dk, c0, c1)
                        for c in range(c0, c1):
                            ri = int(lay["chunk_run"][c])
                            if ri not in g_tiles[dk]:
                                emit_run(dk, ri)
                            nc.tensor.matmul(
                                aps[:],
                                g_tiles[dk][ri][:, c - int(lay["run_c0"][ri]), 0:D],
                                m_tiles[dk][c0][:, c - c0, :],
                                start=(j == 0), stop=(j == nchunks - 1))
                            j += 1
                    nc.vector.tensor_copy(at[:], aps[:])
                    agg_t[dk] = at

                ncols = min(128, NPC - b * 128)
                ops = out_pspool.tile([D, ncols], f32, tag="outps",
                                      name=f"ops_{b}")
                nc.tensor.matmul(ops[:], Wt[:, 0:D],
                                 xT[:, b * 128:b * 128 + ncols],
                                 start=True, stop=False)
                nc.tensor.matmul(ops[:], Wt[:, D:2 * D],
                                 agg_t["in"][:, 0:ncols],
                                 start=False, stop=False)
                nc.tensor.matmul(ops[:], Wt[:, 2 * D:3 * D],
                                 agg_t["out"][:, 0:ncols],
                                 start=False, stop=True)
                nc.scalar.activation(
                    r_sb[:, b * 128:b * 128 + ncols], ops[:],
                    mybir.ActivationFunctionType.Relu,
                    accum_out=sums[:, b:b + 1])
                nc.scalar.activation(
                    sq_scr[:, 0:ncols], r_sb[:, b * 128:b * 128 + ncols],
                    mybir.ActivationFunctionType.Square,
                    accum_out=sumsq[:, b:b + 1])

            # --- BN stats allreduce
            part = cpool.tile([D, 2], f32, tag="part")
            nc.vector.tensor_reduce(part[:, 0:1], sums[:],
                                    mybir.AxisListType.X, mybir.AluOpType.add)
            nc.vector.tensor_reduce(part[:, 1:2], sumsq[:],
                                    mybir.AxisListType.X, mybir.AluOpType.add)
            nc.sync.dma_start(cc_in[:], part[:])
            nc.gpsimd.collective_compute(
                "AllReduce", mybir.AluOpType.add,
                replica_groups=[list(range(N_CORES))],
                ins=[cc_in[:]], outs=[cc_out[:]])
            tot = cpool.tile([D, 2], f32, tag="tot")
            nc.sync.dma_start(tot[:], cc_out[:])

            # --- scale/shift
            stats = cpool.tile([D, 8], f32, tag="stats")
            mean, ex2 = stats[:, 0:1], stats[:, 1:2]
            var, std = stats[:, 2:3], stats[:, 3:4]
            inv, scale = stats[:, 4:5], stats[:, 5:6]
            shift, tmp = stats[:, 6:7], stats[:, 7:8]
            inv_n = 1.0 / float(N_NODES)
            nc.vector.tensor_scalar_mul(mean, tot[:, 0:1], inv_n)
            nc.vector.tensor_scalar_mul(ex2, tot[:, 1:2], inv_n)
            nc.vector.tensor_tensor(tmp, mean, mean, op=mybir.AluOpType.mult)
            nc.vector.tensor_tensor(var, ex2, tmp,
                                    op=mybir.AluOpType.subtract)
            nc.vector.tensor_scalar_add(var, var, BN_EPS)
            nc.scalar.activation(std, var, mybir.ActivationFunctionType.Sqrt)
            nc.vector.reciprocal(inv, std)
            nc.vector.tensor_tensor(scale, gb[:, 0:1], inv,
                                    op=mybir.AluOpType.mult)
            nc.vector.tensor_tensor(tmp, mean, scale,
                                    op=mybir.AluOpType.mult)
            nc.vector.tensor_tensor(shift, gb[:, 1:2], tmp,
                                    op=mybir.AluOpType.subtract)

            # --- normalize into the retired xT buffer + writeback
            nc.vector.tensor_scalar(xT[:], r_sb[:], scale, shift,
                                    op0=mybir.AluOpType.mult,
                                    op1=mybir.AluOpType.add)
            nc.sync.dma_start(out_d[:], xT[:])

    if _bcast_failed:
        print(f"note: mask broadcast tensor_tensor fell back to copy "
              f"for {len(_bcast_failed)} batches", file=sys.stderr)
    nc.finalize()
    return nc


# ---------------------------------------------------------------- kernel

def prepare(x, edge_index, num_nodes=None, W_in=None, W_out=None,
            W_self=None, gamma=None, beta=None):
    """Build the bass program and per-core input maps."""
    x = np.asarray(x, np.float32)
    edge_index = np.asarray(edge_index, np.int64)
    W_in = np.asarray(W_in, np.float32)
    W_out = np.asarray(W_out, np.float32)
    W_self = np.asarray(W_self, np.float32)
    gamma = np.asarray(gamma, np.float32)
    beta = np.asarray(beta, np.float32)
    assert x.shape == (N_NODES, D) and edge_index.shape == (2, N_EDGES)

    src, dst = edge_index[0], edge_index[1]
    lay_in, pc_in = _route_direction(dst, src)    # agg_in: reduce by dst
    lay_out, pc_out = _route_direction(src, dst)  # agg_out: reduce by src

    nc = _build_program(lay_in, lay_out)

    x16 = x.astype(np.float16)
    xdup = np.concatenate([x16, x16], axis=1)
    Wt = np.concatenate([W_self.T, W_in.T, W_out.T], axis=1).astype(np.float32)
    Wt = np.ascontiguousarray(Wt)
    gb = np.ascontiguousarray(np.stack([gamma, beta], axis=1).astype(np.float32))

    in_maps = []
    for c in range(N_CORES):
        xT_c = np.ascontiguousarray(x[c * NPC:(c + 1) * NPC].T)
        in_maps.append({
            "xdup": xdup,
            "xT": xT_c,
            "Wt": Wt,
            "gb": gb,
            "idx_in": pc_in[c][0], "dcmp_in": pc_in[c][1],
            "idx_out": pc_out[c][0], "dcmp_out": pc_out[c][1],
        })
    return nc, in_maps


def postprocess(results):
    outT = np.concatenate([r["out"] for r in results], axis=1)
    return np.ascontiguousarray(outT.T).astype(np.float32)


def kernel(x, edge_index, num_nodes=None, W_in=None, W_out=None,
           W_self=None, gamma=None, beta=None):
    from concourse.bass_utils import run_bass_kernel_spmd

    nc, in_maps = prepare(x, edge_index, num_nodes, W_in, W_out,
                          W_self, gamma, beta)
    res = run_bass_kernel_spmd(nc, in_maps, core_ids=list(range(N_CORES)))
    return postprocess(res.results)


# revision 21
# speedup vs baseline: 2.2946x; 1.1982x over previous
"""DGCNConv (GNN message passing) Trainium2 kernel, 8-core SPMD.

Strategy (graph/data parallel, per sharding hint):
- Nodes are partitioned into 8 contiguous ranges of 6250. Core c owns the
  dst-range edges for agg_in and the src-range edges for agg_out.
- Per direction, edges are sorted HALF-MAJOR (gather-table half, then target
  block of 128): each direction becomes two giant contiguous gather streams,
  so SWDGE dma_gather instructions are maximal-size (GCAP) and their ~1us
  fixed issue overhead is amortized (the dominant cost of the previous
  block-major layout).
- Edges are gathered from HBM with SWDGE dma_gather (256B rows, fp16
  duplicated x table) and segment-reduced on-chip with one-hot matmuls
  accumulating in PSUM (aggT layout [feat, node]).
- Per 128-node block (interleaved, not phase-by-phase): reduce in-dir
  chunks, reduce out-dir chunks, then immediately run the epilogue
  outT = W_self@xT + W_in@agg_inT + W_out@agg_outT, ReLU (+BN partial
  sums). This overlaps PE/Act/DVE with the gather stream and avoids
  materializing full-size agg buffers (SBUF was the binding constraint).
- Tail: cross-core AllReduce of BN partials, fused scale/shift normalize
  (written into the retired xT buffer), writeback.
- Host: routes/pads edges, builds index tables, transposes the output back.

The per-(half,block) segment sizes are padded to the max over cores so all
8 cores run one identical program (SPMD NEFF) on per-core data.
"""

import sys

if "/opt/trn_rl_repo" not in sys.path:
    sys.path.insert(0, "/opt/trn_rl_repo")

import numpy as np

N_NODES = 50000
N_EDGES = 800000
D = 64
N_CORES = 8
NPC = N_NODES // N_CORES          # 6250 nodes per core
NBLK = (NPC + 127) // 128         # 49 blocks per core
HALF = N_NODES // 2               # 25000, int16-safe gather base split
BN_EPS = 1e-5
GCAP = 1024                       # positions per dma_gather instruction
SCRATCH = 16384                   # SWDGE descriptor ring bytes (16B/desc)
MB = 16                           # 128-edge chunks per one-hot build batch


# ---------------------------------------------------------------- host prep

def _route_direction(t_all, g_all):
    """Route edges (t = reduce-target node id, g = gather node id) to cores.

    Returns (layout, per_core): layout is the static position map shared by
    all cores (half-major streams); per_core holds each core's idx/dcmp.
    """
    core_of = t_all // NPC
    per_core_edges = []
    for c in range(N_CORES):
        m = core_of == c
        t = t_all[m] - c * NPC
        g = g_all[m]
        blk = t >> 7
        half = (g >= HALF).astype(np.int64)
        order = np.lexsort((g, blk, half))
        per_core_edges.append((t[order], g[order], blk[order], half[order]))

    # static budgets per (half, blk): max over cores, padded to whole chunks
    budgets = np.zeros((2, NBLK), np.int64)
    for c in range(N_CORES):
        t, g, blk, half = per_core_edges[c]
        cnt = np.bincount(half * NBLK + blk, minlength=2 * NBLK).reshape(2, NBLK)
        budgets = np.maximum(budgets, cnt)
    budgets = ((budgets + 127) // 128) * 128

    # layout: positions ordered half-major, then block
    seg_start = np.zeros((2, NBLK), np.int64)
    pos = 0
    for h in range(2):
        for b in range(NBLK):
            seg_start[h, b] = pos
            pos += budgets[h, b]
    total = pos
    nch = total // 128

    # gather runs: each half stream is contiguous; split at GCAP only
    runs = []  # (pos0, npos, half)
    for h in range(2):
        p0 = int(seg_start[h, 0])
        pend = int(seg_start[h, NBLK - 1] + budgets[h, NBLK - 1])
        while p0 < pend:
            take = min(pend - p0, GCAP)
            runs.append((p0, take, h))
            p0 += take

    # per-block chunk ranges (one per half)
    blocks = []  # (blk, [(c0, c1), ...])
    for b in range(NBLK):
        segs = []
        for h in range(2):
            c0 = int(seg_start[h, b]) // 128
            c1 = c0 + int(budgets[h, b]) // 128
            if c1 > c0:
                segs.append((c0, c1))
        blocks.append((b, segs))

    # chunk -> run mapping
    chunk_run = np.zeros(nch, np.int64)
    run_c0 = np.zeros(len(runs), np.int64)
    for ri, (p0, n, h) in enumerate(runs):
        run_c0[ri] = p0 // 128
        chunk_run[p0 // 128:(p0 + n) // 128] = ri

    # per-core position arrays
    per_core = []
    for c in range(N_CORES):
        t, g, blk, half = per_core_edges[c]
        idx = np.zeros(total, np.int16)          # gather idx rel to half base
        dcmp = np.full(total, -1.0, np.float16)  # one-hot compare value
        key = half * NBLK + blk
        cnt = np.bincount(key, minlength=2 * NBLK)
        estart = np.zeros(2 * NBLK, np.int64)
        estart[1:] = np.cumsum(cnt)[:-1]
        for h in range(2):
            for b in range(NBLK):
                n = int(cnt[h * NBLK + b])
                if n == 0:
                    continue
                e0 = int(estart[h * NBLK + b])
                p0 = int(seg_start[h, b])
                idx[p0:p0 + n] = (g[e0:e0 + n] - h * HALF).astype(np.int16)
                dcmp[p0:p0 + n] = (t[e0:e0 + n] - b * 128).astype(np.float16)
        idx_wrapped = np.tile(
            np.ascontiguousarray(idx.reshape(-1, 16).T), (8, 1))
        dcmp_wrapped = np.ascontiguousarray(dcmp.reshape(-1, 128).T)
        per_core.append((idx_wrapped, dcmp_wrapped))

    layout = dict(total=total, nch=nch, runs=runs, blocks=blocks,
                  chunk_run=chunk_run, run_c0=run_c0)
    return layout, per_core


# ---------------------------------------------------------------- program

def _build_program(lay_in, lay_out):
    import concourse.bacc as bacc
    import concourse.mybir as mybir
    from concourse import tile
    from concourse import library_config

    f32, f16, i16 = mybir.dt.float32, mybir.dt.float16, mybir.dt.int16
    nc = bacc.Bacc(None, target_bir_lowering=False, debug=False,
                   dynamic_dma_scratch_size=SCRATCH, num_swdge_queues=4)

    xdup = nc.dram_tensor("xdup", [N_NODES, 2 * D], f16, kind="ExternalInput")
    xT_d = nc.dram_tensor("xT", [D, NPC], f32, kind="ExternalInput")
    Wt_d = nc.dram_tensor("Wt", [D, 3 * D], f32, kind="ExternalInput")
    gb_d = nc.dram_tensor("gb", [D, 2], f32, kind="ExternalInput")
    out_d = nc.dram_tensor("out", [D, NPC], f32, kind="ExternalOutput")
    cc_in = nc.dram_tensor("cc_in", [D, 2], f32)
    cc_out = nc.dram_tensor("cc_out", [D, 2], f32, addr_space="Shared")

    lays = {"in": lay_in, "out": lay_out}
    idx_d, dcmp_d = {}, {}
    for dk in ("in", "out"):
        tot = lays[dk]["total"]
        idx_d[dk] = nc.dram_tensor(
            f"idx_{dk}", [128, tot // 16], i16, kind="ExternalInput")
        dcmp_d[dk] = nc.dram_tensor(
            f"dcmp_{dk}", [128, tot // 128], f16, kind="ExternalInput")

    with tile.TileContext(nc) as tc:
        nc.gpsimd.load_library(library_config.mlp)
        with (
            tc.tile_pool(name="const", bufs=1) as cpool,
            tc.tile_pool(name="gath", bufs=24) as gpool,
            tc.tile_pool(name="mb", bufs=10) as mpool,
            tc.tile_pool(name="dr", bufs=4) as dpool,
            tc.tile_pool(name="aggt", bufs=4) as apool,
            tc.tile_pool(name="agg_ps", bufs=4, space="PSUM") as agg_pspool,
            tc.tile_pool(name="out_ps", bufs=3, space="PSUM") as out_pspool,
        ):
            # --- constants
            xT = cpool.tile([D, NPC], f32, tag="xT")
            nc.sync.dma_start(xT[:], xT_d[:])
            Wt = cpool.tile([D, 3 * D], f32, tag="Wt")
            nc.sync.dma_start(Wt[:], Wt_d[:])
            gb = cpool.tile([D, 2], f32, tag="gb")
            nc.sync.dma_start(gb[:], gb_d[:])
            iota_i = cpool.tile([128, MB, 128], i16, tag="iota_i")
            nc.gpsimd.iota(iota_i[:], [[0, MB], [1, 128]], base=0,
                           channel_multiplier=0)
            iota_f = cpool.tile([128, MB, 128], f16, tag="iota_f")
            nc.vector.tensor_copy(iota_f[:], iota_i[:])

            idx_t, dcmp_t = {}, {}
            for dk in ("in", "out"):
                tot = lays[dk]["total"]
                idx_t[dk] = cpool.tile([128, tot // 16], i16, tag=f"idx{dk}",
                                       name=f"idx_t_{dk}")
                nc.sync.dma_start(idx_t[dk][:], idx_d[dk][:])
                dcmp_t[dk] = cpool.tile([128, tot // 128], f16, tag=f"dc{dk}",
                                        name=f"dcmp_t_{dk}")
                nc.sync.dma_start(dcmp_t[dk][:], dcmp_d[dk][:])

            g_tiles = {"in": {}, "out": {}}
            m_tiles = {"in": {}, "out": {}}
            _bcast_failed = []

            def emit_run(dk, ri):
                lay = lays[dk]
                p0, npos, h = lay["runs"][ri]
                gt = gpool.tile([128, npos // 128, 2 * D], f16, tag="g",
                                name=f"g_{dk}_{ri}")
                src = xdup[h * HALF:(h + 1) * HALF, :]
                qn = (0 if dk == "in" else 2) + h  # one queue per stream
                nc.gpsimd.dma_gather(
                    gt[:], src, idx_t[dk][:, p0 // 16:(p0 + npos) // 16],
                    npos, npos, 2 * D, queue_num=qn)
                g_tiles[dk][ri] = gt

            def emit_seg(dk, c0, c1):
                """Build the one-hot masks for one (half, block) segment."""
                nb = c1 - c0
                assert nb <= MB, (c0, c1)
                bcast = dcmp_t[dk][:, c0:c0 + nb].unsqueeze(2) \
                    .broadcast_to([128, nb, 128])
                mt = mpool.tile([128, nb, 128], f16, tag="m",
                                name=f"m_{dk}_{c0}")
                try:
                    nc.vector.tensor_tensor(
                        mt[:], iota_f[:, :nb, :], bcast,
                        op=mybir.AluOpType.is_equal)
                except Exception:
                    _bcast_failed.append(c0)
                    dr = dpool.tile([128, nb, 128], f16, tag="drep",
                                    name=f"dr_{dk}_{c0}")
                    nc.vector.tensor_copy(dr[:], bcast)
                    nc.vector.tensor_tensor(
                        mt[:], iota_f[:, :nb, :], dr[:],
                        op=mybir.AluOpType.is_equal)
                m_tiles[dk][c0] = mt

            # --- per-block: segment-reduce both directions, then epilogue
            r_sb = cpool.tile([D, NPC], f32, tag="r")
            sums = cpool.tile([D, NBLK], f32, tag="sums")
            sumsq = cpool.tile([D, NBLK], f32, tag="sumsq")
            sq_scr = cpool.tile([D, 128], f32, tag="sq")

            for b in range(NBLK):
                agg_t = {}
                for dk in ("in", "out"):
                    lay = lays[dk]
                    _, segs = lay["blocks"][b]
                    at = apool.tile([D, 128], f32, tag="aggt",
                                    name=f"at_{dk}_{b}")
                    if not segs:
                        nc.vector.memset(at[:], 0.0)
                        agg_t[dk] = at
                        continue
                    nchunks = sum(c1 - c0 for (c0, c1) in segs)
                    aps = agg_pspool.tile([D, 128], f32, tag="aggps",
                                          name=f"aps_{dk}_{b}")
                    j = 0
                    for (c0, c1) in segs:
                        emit_seg(dk, c0, c1)
                        for c in range(c0, c1):
                            ri = int(lay["chunk_run"][c])
                            if ri not in g_tiles[dk]:
                                emit_run(dk, ri)
                            nc.tensor.matmul(
                                aps[:],
                                g_tiles[dk][ri][:, c - int(lay["run_c0"][ri]), 0:D],
                                m_tiles[dk][c0][:, c - c0, :],
                                start=(j == 0), stop=(j == nchunks - 1))
                            j += 1
                    nc.vector.tensor_copy(at[:], aps[:])
                    agg_t[dk] = at

                ncols = min(128, NPC - b * 128)
                ops = out_pspool.tile([D, ncols], f32, tag="outps",
                                      name=f"ops_{b}")
                nc.tensor.matmul(ops[:], Wt[:, 0:D],
                                 xT[:, b * 128:b * 128 + ncols],
                                 start=True, stop=False)
                nc.tensor.matmul(ops[:], Wt[:, D:2 * D],
                                 agg_t["in"][:, 0:ncols],
                                 start=False, stop=False)
                nc.tensor.matmul(ops[:], Wt[:, 2 * D:3 * D],
                                 agg_t["out"][:, 0:ncols],
                                 start=False, stop=True)
                nc.scalar.activation(
                    r_sb[:, b * 128:b * 128 + ncols], ops[:],
                    mybir.ActivationFunctionType.Relu,
                    accum_out=sums[:, b:b + 1])
                nc.scalar.activation(
                    sq_scr[:, 0:ncols], r_sb[:, b * 128:b * 128 + ncols],
                    mybir.ActivationFunctionType.Square,
                    accum_out=sumsq[:, b:b + 1])

            # --- BN stats allreduce
            part = cpool.tile([D, 2], f32, tag="part")
            nc.vector.tensor_reduce(part[:, 0:1], sums[:],
                                    mybir.AxisListType.X, mybir.AluOpType.add)
            nc.vector.tensor_reduce(part[:, 1:2], sumsq[:],
                                    mybir.AxisListType.X, mybir.AluOpType.add)
            nc.sync.dma_start(cc_in[:], part[:])
            nc.gpsimd.collective_compute(
                "AllReduce", mybir.AluOpType.add,
                replica_groups=[list(range(N_CORES))],
                ins=[cc_in[:]], outs=[cc_out[:]])
            tot = cpool.tile([D, 2], f32, tag="tot")
            nc.sync.dma_start(tot[:], cc_out[:])

            # --- scale/shift
            stats = cpool.tile([D, 8], f32, tag="stats")
            mean, ex2 = stats[:, 0:1], stats[:, 1:2]
            var, std = stats[:, 2:3], stats[:, 3:4]
            inv, scale = stats[:, 4:5], stats[:, 5:6]
            shift, tmp = stats[:, 6:7], stats[:, 7:8]
            inv_n = 1.0 / float(N_NODES)
            nc.vector.tensor_scalar_mul(mean, tot[:, 0:1], inv_n)
            nc.vector.tensor_scalar_mul(ex2, tot[:, 1:2], inv_n)
            nc.vector.tensor_tensor(tmp, mean, mean, op=mybir.AluOpType.mult)
            nc.vector.tensor_tensor(var, ex2, tmp,
                                    op=mybir.AluOpType.subtract)
            nc.vector.tensor_scalar_add(var, var, BN_EPS)
            nc.scalar.activation(std, var, mybir.ActivationFunctionType.Sqrt)
            nc.vector.reciprocal(inv, std)
            nc.vector.tensor_tensor(scale, gb[:, 0:1], inv,
                                    op=mybir.AluOpType.mult)
            nc.vector.tensor_tensor(tmp, mean, scale,
                                    op=mybir.AluOpType.mult)
            nc.vector.tensor_tensor(shift, gb[:, 1:2], tmp,
                                    op=mybir.AluOpType.subtract)

            # --- normalize into the retired xT buffer + writeback
            nc.vector.tensor_scalar(xT[:], r_sb[:], scale, shift,
                                    op0=mybir.AluOpType.mult,
                                    op1=mybir.AluOpType.add)
            nc.sync.dma_start(out_d[:], xT[:])

    if _bcast_failed:
        print(f"note: mask broadcast tensor_tensor fell back to copy "
              f"for {len(_bcast_failed)} batches", file=sys.stderr)
    nc.finalize()
    return nc


# ---------------------------------------------------------------- kernel

def prepare(x, edge_index, num_nodes=None, W_in=None, W_out=None,
            W_self=None, gamma=None, beta=None):
    """Build the bass program and per-core input maps."""
    x = np.asarray(x, np.float32)
    edge_index = np.asarray(edge_index, np.int64)
    W_in = np.asarray(W_in, np.float32)
    W_out = np.asarray(W_out, np.float32)
    W_self = np.asarray(W_self, np.float32)
    gamma = np.asarray(gamma, np.float32)
    beta = np.asarray(beta, np.float32)
    assert x.shape == (N_NODES, D) and edge_index.shape == (2, N_EDGES)

    src, dst = edge_index[0], edge_index[1]
    lay_in, pc_in = _route_direction(dst, src)    # agg_in: reduce by dst
    lay_out, pc_out = _route_direction(src, dst)  # agg_out: reduce by src

    nc = _build_program(lay_in, lay_out)

    x16 = x.astype(np.float16)
    xdup = np.concatenate([x16, x16], axis=1)
    Wt = np.concatenate([W_self.T, W_in.T, W_out.T], axis=1).astype(np.float32)
    Wt = np.ascontiguousarray(Wt)
    gb = np.ascontiguousarray(np.stack([gamma, beta], axis=1).astype(np.float32))

    in_maps = []
    for c in range(N_CORES):
        xT_c = np.ascontiguousarray(x[c * NPC:(c + 1) * NPC].T)
        in_maps.append({
            "xdup": xdup,
            "xT": xT_c,
            "Wt": Wt,
            "gb": gb,
            "idx_in": pc_in[c][0], "dcmp_in": pc_in[c][1],
            "idx_out": pc_out[c][0], "dcmp_out": pc_out[c][1],
        })
    return nc, in_maps


def postprocess(results):
    outT = np.concatenate([r["out"] for r in results], axis=1)
    return np.ascontiguousarray(outT.T).astype(np.float32)


def kernel(x, edge_index, num_nodes=None, W_in=None, W_out=None,
           W_self=None, gamma=None, beta=None):
    from concourse.bass_utils import run_bass_kernel_spmd

    nc, in_maps = prepare(x, edge_index, num_nodes, W_in, W_out,
                          W_self, gamma, beta)
    res = run_bass_kernel_spmd(nc, in_maps, core_ids=list(range(N_CORES)))
    return postprocess(res.results)
